# revision 20
# baseline (speedup 1.0000x reference)
"""Trainium2 Bass kernel for the NumReps masked-mean problem.

Math: each mask row is a contiguous run of ones (1..8 long). expand_window
widens it by int(0.2*len) (== 1 iff len >= 5) on each side, clamped to
[0, S-1]; the output row is the mean of reps rows over the widened window
(window length n <= 10, and n is never 5 or 6).

Strategy (per core, data-parallel over batch: 16 batches / 8 cores = 2):
  - two fused scalar_tensor_tensor passes over the mask give
    sum((iota-4096)*mask) and sum((iota+4096)*mask) -> run length and
    position-sum -> first/last index, recovered exactly (rint trick)
  - derive window start ns, length n, weight 1/n with tiny [128,1] ops
  - indirect-DMA gather of the window rows, split 5+5: the first five
    rows always (one 20KB descriptor per output row), the last five only
    for rows with n >= 7 (OOB-skip trick; the landing area is pre-zeroed
    so skipped rows contribute exact zeros)
  - weighted windowed sum on the TensorEngine: 10 accumulating diagonal
    matmuls in float32r (full rate at N=512), diag_j = diag((j<n)/n)
  - PSUM -> SBUF copy, store
"""

import numpy as np

B, M, S, D = 16, 128, 2048, 1024
NCORES = 8
BPC = B // NCORES  # batches per core
WMAX = 10  # max expanded window length
G1 = 4  # chunks in the unconditional gather (n is 1..4 or 7..10; the
        # conditional gather covers chunks G1..9 for n >= 7 rows only)
RINT_MAGIC = 12582912.0  # 2^23 + 2^22: (x + magic) - magic == rint(x) for |x| < 2^22

# weighted-reduce engine: "pe_f32r" (fast) | "pe_f32" | "dve"
REDUCE_MODE = "pe_f32r"
SPLIT_GATHER = True

_cache = {}


def _build_nc():
    import concourse.bacc as bacc
    import concourse.bass as bass
    import concourse.mybir as mybir
    from concourse import tile

    f32 = mybir.dt.float32
    f32r = mybir.dt.float32r
    i32 = mybir.dt.int32
    Alu = mybir.AluOpType
    Axis = mybir.AxisListType

    nc = bacc.Bacc("TRN2", target_bir_lowering=False, debug=False)

    mask = nc.dram_tensor("mask", [BPC, M, S], f32, kind="ExternalInput")
    reps = [
        nc.dram_tensor(f"reps{b}", [S, D], f32, kind="ExternalInput")
        for b in range(BPC)
    ]
    out = nc.dram_tensor("out", [BPC, M, D], f32, kind="ExternalOutput")

    iota_np = np.broadcast_to(
        np.arange(S, dtype=np.float32), (M, S)
    )
    iota_const = nc.inline_tensor(np.ascontiguousarray(iota_np), name="iota_const")

    with tile.TileContext(nc) as tc:
        with (
            tc.tile_pool(name="const", bufs=1) as cpool,
            tc.tile_pool(name="big", bufs=2) as big,
            tc.tile_pool(name="small", bufs=2) as small,
            tc.tile_pool(name="psum", bufs=2, space="PSUM") as psum,
        ):
            # constant: iota 0..S-1 (same in every partition), DMA'd from an
            # inline NEFF constant (overlaps the mask loads)
            iota_f = cpool.tile([M, S], f32)
            nc.sync.dma_start(iota_f[:], iota_const[:])

            # explicit gather tiles (one pair per batch): gta takes the
            # unconditional first-G1-chunks gather, gtb the conditional tail
            # (separate tiles so the tail memset/gather never serialize
            # against the first gather or its casts)
            gtas = [
                cpool.tile([M, G1 * D], f32, tag=f"gta{b}", name=f"gta{b}")
                for b in range(BPC)
            ]
            gtbs = [
                cpool.tile([M, (WMAX - G1) * D], f32, tag=f"gtb{b}", name=f"gtb{b}")
                for b in range(BPC)
            ]
            if SPLIT_GATHER:
                for b in range(BPC):
                    # gpsimd is idle early; separate tiles keep this off the
                    # gather/cast dependency chains
                    nc.gpsimd.memset(gtbs[b][:], 0.0)

            for b in range(BPC):
                mt = big.tile([M, S], f32, tag="mask")
                nc.sync.dma_start(mt[:], mask[b])

                # len on the scalar engine: accum_out = sum(mask)
                p = big.tile([M, S], f32, tag="p")
                lenf = small.tile([M, 1], f32, tag="lenf")
                nc.scalar.activation(
                    out=p[:], in_=mt[:],
                    func=mybir.ActivationFunctionType.Identity,
                    accum_out=lenf[:],
                )
                # one DVE pass: A1 = sum((iota-4096)*mask) = possum - 4096*len
                p2 = big.tile([M, S], f32, tag="p")
                a1 = small.tile([M, 1], f32, tag="a1")
                nc.vector.scalar_tensor_tensor(
                    out=p2[:], in0=iota_f[:], scalar=-4096.0, in1=mt[:],
                    op0=Alu.add, op1=Alu.mult, accum_out=a1[:],
                )
                # possum = A1 + 4096*len (exact)
                psm = small.tile([M, 1], f32, tag="psm")
                nc.vector.tensor_scalar(
                    out=psm[:], in0=lenf[:], scalar1=4096.0,
                    scalar2=a1[:, :1], op0=Alu.mult, op1=Alu.add,
                )

                # first = rint(possum/len - (len-1)/2)
                rl = small.tile([M, 1], f32, tag="rl")
                nc.vector.reciprocal(rl[:], lenf[:])
                half_lm1 = small.tile([M, 1], f32, tag="hlm1")
                nc.vector.tensor_scalar(
                    out=half_lm1[:], in0=lenf[:], scalar1=-1.0, scalar2=0.5,
                    op0=Alu.add, op1=Alu.mult,
                )
                first = small.tile([M, 1], f32, tag="first")
                nc.vector.tensor_scalar(
                    out=first[:], in0=psm[:], scalar1=rl[:, :1],
                    scalar2=half_lm1[:, :1], op0=Alu.mult, op1=Alu.subtract,
                )
                nc.vector.tensor_scalar(
                    out=first[:], in0=first[:], scalar1=RINT_MAGIC,
                    scalar2=-RINT_MAGIC, op0=Alu.add, op1=Alu.add,
                )
                last = small.tile([M, 1], f32, tag="last")
                nc.vector.tensor_scalar(
                    out=last[:], in0=first[:], scalar1=lenf[:, :1],
                    scalar2=-1.0, op0=Alu.add, op1=Alu.add,
                )

                # expand = 1 iff len >= 5
                e = small.tile([M, 1], f32, tag="e")
                nc.vector.tensor_scalar(
                    out=e[:], in0=lenf[:], scalar1=4.5, scalar2=None,
                    op0=Alu.is_ge,
                )
                # ns = max(first-e, 0); ne = min(last+e, S-1); n = ne-ns+1
                ns = small.tile([M, 1], f32, tag="ns")
                nc.vector.tensor_scalar(
                    out=ns[:], in0=first[:], scalar1=e[:, :1], scalar2=0.0,
                    op0=Alu.subtract, op1=Alu.max,
                )
                ne = small.tile([M, 1], f32, tag="ne")
                nc.vector.tensor_scalar(
                    out=ne[:], in0=last[:], scalar1=e[:, :1],
                    scalar2=float(S - 1), op0=Alu.add, op1=Alu.min,
                )
                n = small.tile([M, 1], f32, tag="n")
                nc.vector.tensor_scalar(
                    out=n[:], in0=ne[:], scalar1=ns[:, :1], scalar2=1.0,
                    op0=Alu.subtract, op1=Alu.add,
                )
                inv = small.tile([M, 1], f32, tag="inv")
                nc.vector.reciprocal(inv[:], n[:])
                nsi = small.tile([M, 1], i32, tag="nsi")
                nc.vector.tensor_copy(nsi[:], ns[:])

                # weights: w[m, j] = (j < n_m) * inv_m     [M, WMAX]
                w = small.tile([M, WMAX], f32, tag="w")
                nc.vector.tensor_scalar(
                    out=w[:], in0=iota_f[:, :WMAX], scalar1=n[:, :1],
                    scalar2=inv[:, :1], op0=Alu.is_lt, op1=Alu.mult,
                )

                # gather: chunk j of row m = reps[b][ns_m + j, :]
                gta, gtb = gtas[b], gtbs[b]
                nc.gpsimd.indirect_dma_start(
                    out=gta[:],
                    out_offset=None,
                    in_=reps[b][:],
                    in_offset=bass.IndirectOffsetOnAxis(ap=nsi[:, :1], axis=0),
                )
                # tail chunks only for n >= 7 rows (e == 1); others get an
                # out-of-range index and are skipped (tail stays zero)
                idx2 = small.tile([M, 1], f32, tag="idx2")
                nc.vector.tensor_scalar(
                    out=idx2[:], in0=e[:], scalar1=-4096.0,
                    scalar2=ns[:, :1], op0=Alu.mult, op1=Alu.add,
                )
                nc.vector.tensor_scalar_add(idx2[:], idx2[:], 4096.0 + G1)
                nsi2 = small.tile([M, 1], i32, tag="nsi2")
                nc.vector.tensor_copy(nsi2[:], idx2[:])
                nc.gpsimd.indirect_dma_start(
                    out=gtb[:],
                    out_offset=None,
                    in_=reps[b][:],
                    in_offset=bass.IndirectOffsetOnAxis(ap=nsi2[:, :1], axis=0),
                    bounds_check=S - 1,
                    oob_is_err=False,
                )

                osum = big.tile([M, D], f32, tag="osum")
                if REDUCE_MODE.startswith("pe"):
                    mm_dt = f32r if REDUCE_MODE == "pe_f32r" else f32
                    # diag_j = diag(w[:, j]), all WMAX blocks in one
                    # affine_select: iota[p, j, f] = p - f, keep w[p, j]
                    # where f == p, else 0
                    diag = big.tile([M, WMAX * M], mm_dt, tag="diag")
                    nc.gpsimd.affine_select(
                        out=diag[:].rearrange("p (j q) -> p j q", j=WMAX),
                        in_=w[:].unsqueeze(-1).to_broadcast([M, WMAX, M]),
                        compare_op=Alu.is_equal,
                        fill=0.0,
                        base=0,
                        pattern=[[0, WMAX], [-1, M]],
                        channel_multiplier=1,
                    )
                    if REDUCE_MODE == "pe_f32r":
                        # fp32r inputs must be produced by a rounding op (the
                        # verifier keys on the memory location, so the
                        # DMA-written gather tiles can't feed the PE
                        # directly): stage through f32r tiles, head on DVE,
                        # tail on gpsimd (runs concurrently)
                        gtra = big.tile([M, G1 * D], f32r, tag="gtra", bufs=1)
                        nc.vector.tensor_copy(gtra[:], gta[:])
                        gtrb = big.tile([M, (WMAX - G1) * D], f32r,
                                        tag="gtrb", bufs=1)
                        nc.gpsimd.tensor_copy(gtrb[:], gtb[:])

                        def rhs_pair(j):
                            src = (gtra[:, j * D:(j + 1) * D] if j < G1
                                   else gtrb[:, (j - G1) * D:(j - G1 + 1) * D])
                            return src[:, :512], src[:, 512:]
                    else:
                        def rhs_pair(j):
                            src = (gta[:, j * D:(j + 1) * D] if j < G1
                                   else gtb[:, (j - G1) * D:(j - G1 + 1) * D])
                            return (src[:, :512].bitcast(mm_dt),
                                    src[:, 512:].bitcast(mm_dt))

                    ps0 = psum.tile([M, 512], f32, tag="ps0")
                    ps1 = psum.tile([M, 512], f32, tag="ps1")
                    for j in range(WMAX):
                        dj = diag[:, j * M:(j + 1) * M]
                        rhs0, rhs1 = rhs_pair(j)
                        nc.tensor.matmul(
                            ps0[:], lhsT=dj, rhs=rhs0,
                            start=(j == 0), stop=(j == WMAX - 1),
                        )
                        nc.tensor.matmul(
                            ps1[:], lhsT=dj, rhs=rhs1,
                            start=(j == 0), stop=(j == WMAX - 1),
                        )
                    nc.vector.tensor_copy(osum[:, :512], ps0[:])
                    nc.scalar.copy(osum[:, 512:], ps1[:])
                nc.sync.dma_start(out[b], osum[:])

    nc.finalize()
    return nc


def _get_nc():
    if "nc" not in _cache:
        _cache["nc"] = _build_nc()
    return _cache["nc"]


def _shard_inputs(number_mask, reps):
    in_maps = []
    for c in range(NCORES):
        m = {"mask": np.ascontiguousarray(number_mask[c * BPC:(c + 1) * BPC])}
        for b in range(BPC):
            m[f"reps{b}"] = np.ascontiguousarray(reps[c * BPC + b])
        in_maps.append(m)
    return in_maps


def _install_ntff_hook():
    """The image's antenv lacks axon_hooks; synthesize it so trace=True
    (NTFF profiling) works through run_bass_kernel_spmd."""
    import sys
    import types

    try:
        from antenv.axon_hooks import get_axon_ntff_profile_hook  # noqa: F401
        return
    except ImportError:
        pass
    from trn_agent_boot.trn_boot import _ntff_profile_via_ctypes

    mod = types.ModuleType("antenv.axon_hooks")
    _hook = [_ntff_profile_via_ctypes("/opt/axon/libaxon_pjrt.so")]
    mod.get_axon_ntff_profile_hook = lambda: _hook[0]
    mod.set_axon_ntff_profile_hook = lambda h: _hook.__setitem__(0, h)
    sys.modules["antenv.axon_hooks"] = mod
    import antenv

    antenv.axon_hooks = mod


def _run(number_mask, reps, trace=False):
    from concourse.bass_utils import run_bass_kernel_spmd

    if trace:
        _install_ntff_hook()
    nc = _get_nc()
    in_maps = _shard_inputs(number_mask, reps)
    res = run_bass_kernel_spmd(
        nc, in_maps, core_ids=list(range(NCORES)), trace=trace
    )
    outs = np.stack([r["out"] for r in res.results], axis=0)
    return outs.reshape(B, M, D), res


def kernel(**inputs):
    out, _ = _run(inputs["number_mask"], inputs["reps"], trace=False)
    return out


# revision 21
# speedup vs baseline: 1.7616x; 1.7616x over previous
"""Trainium2 Bass kernel for the NumReps masked-mean problem.

Math: each mask row is a contiguous run of ones (1..8 long). expand_window
widens it by int(0.2*len) (== 1 iff len >= 5) on each side, clamped to
[0, S-1]; the output row is the mean of reps rows over the widened window
(window length n <= 10, and n is never 5 or 6).

Strategy (per core, data-parallel over batch: 16 batches / 8 cores = 2):
  - two fused scalar_tensor_tensor passes over the mask give
    sum((iota-4096)*mask) and sum((iota+4096)*mask) -> run length and
    position-sum -> first/last index, recovered exactly (rint trick)
  - derive window start ns, length n, weight 1/n with tiny [128,1] ops
  - indirect-DMA gather of the window rows, split 5+5: the first five
    rows always (one 20KB descriptor per output row), the last five only
    for rows with n >= 7 (OOB-skip trick; the landing area is pre-zeroed
    so skipped rows contribute exact zeros)
  - weighted windowed sum on the TensorEngine: 10 accumulating diagonal
    matmuls in float32r (full rate at N=512), diag_j = diag((j<n)/n)
  - PSUM -> SBUF copy, store
"""

import numpy as np

B, M, S, D = 16, 128, 2048, 1024
NCORES = 8
BPC = B // NCORES  # batches per core
WMAX = 10  # max expanded window length
G1 = 4  # chunks in the unconditional gather (n is 1..4 or 7..10; the
        # conditional gather covers chunks G1..9 for n >= 7 rows only)
RINT_MAGIC = 12582912.0  # 2^23 + 2^22: (x + magic) - magic == rint(x) for |x| < 2^22

# weighted-reduce engine: "pe_f32r" (fast) | "pe_f32" | "dve"
REDUCE_MODE = "pe_f32r"
SPLIT_GATHER = True

_cache = {}


def _build_nc():
    import concourse.bacc as bacc
    import concourse.bass as bass
    import concourse.mybir as mybir
    from concourse import tile

    f32 = mybir.dt.float32
    f32r = mybir.dt.float32r
    i32 = mybir.dt.int32
    Alu = mybir.AluOpType
    Axis = mybir.AxisListType

    nc = bacc.Bacc("TRN2", target_bir_lowering=False, debug=False)

    mask = nc.dram_tensor("mask", [BPC, M, S], f32, kind="ExternalInput")
    reps = [
        nc.dram_tensor(f"reps{b}", [S, D], f32, kind="ExternalInput")
        for b in range(BPC)
    ]
    out = nc.dram_tensor("out", [BPC, M, D], f32, kind="ExternalOutput")

    iota_np = np.broadcast_to(
        np.arange(S, dtype=np.float32), (M, S)
    )
    iota_const = nc.inline_tensor(np.ascontiguousarray(iota_np), name="iota_const")

    with tile.TileContext(nc) as tc:
        with (
            tc.tile_pool(name="const", bufs=1) as cpool,
            tc.tile_pool(name="big", bufs=2) as big,
            tc.tile_pool(name="small", bufs=2) as small,
            tc.tile_pool(name="psum", bufs=2, space="PSUM") as psum,
        ):
            # constant: iota 0..S-1 (same in every partition), DMA'd from an
            # inline NEFF constant (overlaps the mask loads)
            iota_f = cpool.tile([M, S], f32)
            nc.sync.dma_start(iota_f[:], iota_const[:])

            # explicit gather tiles (one pair per batch): gta takes the
            # unconditional first-G1-chunks gather, gtb the conditional tail
            # (separate tiles so the tail memset/gather never serialize
            # against the first gather or its casts)
            gtas = [
                cpool.tile([M, G1 * D], f32, tag=f"gta{b}", name=f"gta{b}")
                for b in range(BPC)
            ]
            gtbs = [
                cpool.tile([M, (WMAX - G1) * D], f32, tag=f"gtb{b}", name=f"gtb{b}")
                for b in range(BPC)
            ]
            if SPLIT_GATHER:
                for b in range(BPC):
                    # gpsimd is idle early; separate tiles keep this off the
                    # gather/cast dependency chains
                    nc.gpsimd.memset(gtbs[b][:], 0.0)

            for b in range(BPC):
                mt = big.tile([M, S], f32, tag="mask")
                nc.sync.dma_start(mt[:], mask[b])

                # len on the scalar engine: accum_out = sum(mask)
                p = big.tile([M, S], f32, tag="p")
                lenf = small.tile([M, 1], f32, tag="lenf")
                nc.scalar.activation(
                    out=p[:], in_=mt[:],
                    func=mybir.ActivationFunctionType.Identity,
                    accum_out=lenf[:],
                )
                # one DVE pass: A1 = sum((iota-4096)*mask) = possum - 4096*len
                p2 = big.tile([M, S], f32, tag="p")
                a1 = small.tile([M, 1], f32, tag="a1")
                nc.vector.scalar_tensor_tensor(
                    out=p2[:], in0=iota_f[:], scalar=-4096.0, in1=mt[:],
                    op0=Alu.add, op1=Alu.mult, accum_out=a1[:],
                )
                # possum = A1 + 4096*len (exact)
                psm = small.tile([M, 1], f32, tag="psm")
                nc.vector.tensor_scalar(
                    out=psm[:], in0=lenf[:], scalar1=4096.0,
                    scalar2=a1[:, :1], op0=Alu.mult, op1=Alu.add,
                )

                # first = rint(possum/len - (len-1)/2)
                rl = small.tile([M, 1], f32, tag="rl")
                nc.vector.reciprocal(rl[:], lenf[:])
                half_lm1 = small.tile([M, 1], f32, tag="hlm1")
                nc.vector.tensor_scalar(
                    out=half_lm1[:], in0=lenf[:], scalar1=-1.0, scalar2=0.5,
                    op0=Alu.add, op1=Alu.mult,
                )
                first = small.tile([M, 1], f32, tag="first")
                nc.vector.tensor_scalar(
                    out=first[:], in0=psm[:], scalar1=rl[:, :1],
                    scalar2=half_lm1[:, :1], op0=Alu.mult, op1=Alu.subtract,
                )
                nc.vector.tensor_scalar(
                    out=first[:], in0=first[:], scalar1=RINT_MAGIC,
                    scalar2=-RINT_MAGIC, op0=Alu.add, op1=Alu.add,
                )
                last = small.tile([M, 1], f32, tag="last")
                nc.vector.tensor_scalar(
                    out=last[:], in0=first[:], scalar1=lenf[:, :1],
                    scalar2=-1.0, op0=Alu.add, op1=Alu.add,
                )

                # expand = 1 iff len >= 5
                e = small.tile([M, 1], f32, tag="e")
                nc.vector.tensor_scalar(
                    out=e[:], in0=lenf[:], scalar1=4.5, scalar2=None,
                    op0=Alu.is_ge,
                )
                # ns = max(first-e, 0); ne = min(last+e, S-1); n = ne-ns+1
                ns = small.tile([M, 1], f32, tag="ns")
                nc.vector.tensor_scalar(
                    out=ns[:], in0=first[:], scalar1=e[:, :1], scalar2=0.0,
                    op0=Alu.subtract, op1=Alu.max,
                )
                ne = small.tile([M, 1], f32, tag="ne")
                nc.vector.tensor_scalar(
                    out=ne[:], in0=last[:], scalar1=e[:, :1],
                    scalar2=float(S - 1), op0=Alu.add, op1=Alu.min,
                )
                n = small.tile([M, 1], f32, tag="n")
                nc.vector.tensor_scalar(
                    out=n[:], in0=ne[:], scalar1=ns[:, :1], scalar2=1.0,
                    op0=Alu.subtract, op1=Alu.add,
                )
                inv = small.tile([M, 1], f32, tag="inv")
                nc.vector.reciprocal(inv[:], n[:])
                nsi = small.tile([M, 1], i32, tag="nsi")
                nc.vector.tensor_copy(nsi[:], ns[:])

                # weights: w[m, j] = (j < n_m) * inv_m     [M, WMAX]
                w = small.tile([M, WMAX], f32, tag="w")
                nc.vector.tensor_scalar(
                    out=w[:], in0=iota_f[:, :WMAX], scalar1=n[:, :1],
                    scalar2=inv[:, :1], op0=Alu.is_lt, op1=Alu.mult,
                )

                # gather: chunk j of row m = reps[b][ns_m + j, :]
                gta, gtb = gtas[b], gtbs[b]
                nc.gpsimd.indirect_dma_start(
                    out=gta[:],
                    out_offset=None,
                    in_=reps[b][:],
                    in_offset=bass.IndirectOffsetOnAxis(ap=nsi[:, :1], axis=0),
                )
                # tail chunks only for n >= 7 rows (e == 1); others get an
                # out-of-range index and are skipped (tail stays zero)
                idx2 = small.tile([M, 1], f32, tag="idx2")
                nc.vector.tensor_scalar(
                    out=idx2[:], in0=e[:], scalar1=-4096.0,
                    scalar2=ns[:, :1], op0=Alu.mult, op1=Alu.add,
                )
                nc.vector.tensor_scalar_add(idx2[:], idx2[:], 4096.0 + G1)
                nsi2 = small.tile([M, 1], i32, tag="nsi2")
                nc.vector.tensor_copy(nsi2[:], idx2[:])
                nc.gpsimd.indirect_dma_start(
                    out=gtb[:],
                    out_offset=None,
                    in_=reps[b][:],
                    in_offset=bass.IndirectOffsetOnAxis(ap=nsi2[:, :1], axis=0),
                    bounds_check=S - 1,
                    oob_is_err=False,
                )

                osum = big.tile([M, D], f32, tag="osum")
                if REDUCE_MODE.startswith("pe"):
                    mm_dt = f32r if REDUCE_MODE == "pe_f32r" else f32
                    # diag_j = diag(w[:, j]), all WMAX blocks in one
                    # affine_select: iota[p, j, f] = p - f, keep w[p, j]
                    # where f == p, else 0
                    diag = big.tile([M, WMAX * M], mm_dt, tag="diag")
                    nc.gpsimd.affine_select(
                        out=diag[:].rearrange("p (j q) -> p j q", j=WMAX),
                        in_=w[:].unsqueeze(-1).to_broadcast([M, WMAX, M]),
                        compare_op=Alu.is_equal,
                        fill=0.0,
                        base=0,
                        pattern=[[0, WMAX], [-1, M]],
                        channel_multiplier=1,
                    )
                    ps0 = psum.tile([M, 512], f32, tag="ps0")
                    ps1 = psum.tile([M, 512], f32, tag="ps1")
                    for j in range(WMAX):
                        dj = diag[:, j * M:(j + 1) * M]
                        src = (gta[:, j * D:(j + 1) * D] if j < G1
                               else gtb[:, (j - G1) * D:(j - G1 + 1) * D])
                        if REDUCE_MODE == "pe_f32r":
                            # fp32r inputs must be produced by a rounding op
                            # (the verifier keys on the memory location, so
                            # the DMA-written gather tiles can't feed the PE
                            # directly): stage each chunk through f32r tiles
                            # on DVE (gpsimd would contend for the shared
                            # SBUF port)
                            gtr = big.tile([M, D], f32r, tag="gtr", bufs=4,
                                           name=f"gtr_{b}_{j}")
                            nc.vector.tensor_copy(gtr[:], src)
                            rhs0, rhs1 = gtr[:, :512], gtr[:, 512:]
                        else:
                            rhs0 = src[:, :512].bitcast(mm_dt)
                            rhs1 = src[:, 512:].bitcast(mm_dt)
                        nc.tensor.matmul(
                            ps0[:], lhsT=dj, rhs=rhs0,
                            start=(j == 0), stop=(j == WMAX - 1),
                        )
                        nc.tensor.matmul(
                            ps1[:], lhsT=dj, rhs=rhs1,
                            start=(j == 0), stop=(j == WMAX - 1),
                        )
                    nc.vector.tensor_copy(osum[:, :512], ps0[:])
                    nc.scalar.copy(osum[:, 512:], ps1[:])
                nc.sync.dma_start(out[b], osum[:])

    nc.finalize()
    return nc


def _get_nc():
    if "nc" not in _cache:
        _cache["nc"] = _build_nc()
    return _cache["nc"]


def _shard_inputs(number_mask, reps):
    in_maps = []
    for c in range(NCORES):
        m = {"mask": np.ascontiguousarray(number_mask[c * BPC:(c + 1) * BPC])}
        for b in range(BPC):
            m[f"reps{b}"] = np.ascontiguousarray(reps[c * BPC + b])
        in_maps.append(m)
    return in_maps


def _install_ntff_hook():
    """The image's antenv lacks axon_hooks; synthesize it so trace=True
    (NTFF profiling) works through run_bass_kernel_spmd."""
    import sys
    import types

    try:
        from antenv.axon_hooks import get_axon_ntff_profile_hook  # noqa: F401
        return
    except ImportError:
        pass
    from trn_agent_boot.trn_boot import _ntff_profile_via_ctypes

    mod = types.ModuleType("antenv.axon_hooks")
    _hook = [_ntff_profile_via_ctypes("/opt/axon/libaxon_pjrt.so")]
    mod.get_axon_ntff_profile_hook = lambda: _hook[0]
    mod.set_axon_ntff_profile_hook = lambda h: _hook.__setitem__(0, h)
    sys.modules["antenv.axon_hooks"] = mod
    import antenv

    antenv.axon_hooks = mod


def _run(number_mask, reps, trace=False):
    from concourse.bass_utils import run_bass_kernel_spmd

    if trace:
        _install_ntff_hook()
    nc = _get_nc()
    in_maps = _shard_inputs(number_mask, reps)
    res = run_bass_kernel_spmd(
        nc, in_maps, core_ids=list(range(NCORES)), trace=trace
    )
    outs = np.stack([r["out"] for r in res.results], axis=0)
    return outs.reshape(B, M, D), res


def kernel(**inputs):
    out, _ = _run(inputs["number_mask"], inputs["reps"], trace=False)
    return out
